# revision 1
# baseline (speedup 1.0000x reference)
"""CrossMultiheadDiffAttn Trainium2 kernel.

Sharding: 8 cores = 2 batches x 4 head-groups. Each core handles one batch
and 8 of the 32 q/k attention heads (= 4 of the 16 output heads / v-heads).
q/k/v projections column-sharded, out projection row-sharded; the 4 partial
[T, E] outputs per batch are summed on the host.

Per-core device kernel (all matmul operands fp16, fp32 PSUM accumulation):
  A) V = xkv @ Wv (per s-chunk), K^T = Wk^T @ xkv^T, Q^T = Wq^T @ xq^T
     (host pre-transposes x and pre-scales Wq by D^-0.5).
  B) per head-pair, per t-half: scoresT[s,t] = K_h Q_h^T on PE ->
     exp on ACT (no max subtraction; scores ~ N(0,1)) -> P^T fp16 ->
     attn accum [t,64+1] = P^T.T @ [V|1] on PE (ones column gives softmax
     denominators). Pair combine o1/s1 - lam*o2/s2, RMSNorm scale via
     rsqrt(m) = exp(-0.5*ln(m)) (keeps ACT on one table set), transpose
     via PE into attnT.
  C) out^T = Wo'^T @ attnT (Wo' pre-scaled by subln_w*(1-lambda_init)),
     DMA out as [E, T]; host transposes and reduces.
"""

import math

import numpy as np

import concourse.bacc as bacc
import concourse.bass as bass
import concourse.mybir as mybir
import concourse.tile as tile
from concourse import masks
from concourse.bass_utils import run_bass_kernel_spmd
from contextlib import ExitStack

F32 = mybir.dt.float32
F16 = mybir.dt.float16
AF = mybir.ActivationFunctionType
OP = mybir.AluOpType

EMBED = 1024
HEADS = 16
DEPTH = 12
D = 32                       # head dim of q/k heads
LAMBDA_INIT = 0.8 - 0.6 * math.exp(-0.3 * DEPTH)
T = 2048
S = 2048
HC = 256                     # per-core projection columns (8 qk heads * 32)
NE = EMBED // 128            # 8 contraction chunks
NS = S // 128                # 16 s chunks
PAIRS = 4                    # head pairs per core
EPS = 1e-5

_cache = {}

# debug knobs for hardware bisection
DBG_B_PAIRS = 4
DBG_B_AV = True
DBG_B_EPI = True
DBG_B_EPI_LEVEL = 5
DBG_B_TRANS = True


def _av_off(i):
    # 16 psum accumulators of 65 f32 each, packed 7 per 2KB bank (no
    # matmul output may straddle a bank boundary).
    return (i // 7) * 512 + (i % 7) * 65


def _build(lam: float, phases: str = "ABC"):
    nc = bacc.Bacc("TRN2", target_bir_lowering=False, debug=False,
                   enable_asserts=False, num_devices=8)

    xq_d = nc.dram_tensor("xq_t", [EMBED, T], F16, kind="ExternalInput").ap()
    xkv_d = nc.dram_tensor("xkv_t", [EMBED, S], F16, kind="ExternalInput").ap()
    wq_d = nc.dram_tensor("wq", [EMBED, HC], F16, kind="ExternalInput").ap()
    wk_d = nc.dram_tensor("wk", [EMBED, HC], F16, kind="ExternalInput").ap()
    wv_d = nc.dram_tensor("wv", [EMBED, HC], F16, kind="ExternalInput").ap()
    wo_d = nc.dram_tensor("wo", [HC, EMBED], F16, kind="ExternalInput").ap()
    out_d = nc.dram_tensor("out_t", [EMBED, T], F32, kind="ExternalOutput").ap()

    with tile.TileContext(nc) as tc, ExitStack() as ctx:
        persist = ctx.enter_context(tc.tile_pool(name="persist", bufs=1))
        ident = persist.tile([128, 128], F16, tag="ident", name="ident")
        masks.make_identity(nc, ident[:])
        eps_sb = persist.tile([128, 1], F32, tag="eps", name="eps")
        nc.vector.memset(eps_sb[:], EPS)

        qt = [persist.tile([128, T], F16, tag=f"qt{m}", name=f"qt{m}")
              for m in range(2)]
        kt = [persist.tile([128, S], F16, tag=f"kt{m}", name=f"kt{m}")
              for m in range(2)]
        vv = [persist.tile([128, 4 * 65], F16, tag=f"vv{s}", name=f"vv{s}")
              for s in range(NS)]
        use_at = DBG_B_AV and DBG_B_EPI and DBG_B_TRANS
        at = ([persist.tile([128, T], F16, tag=f"at{c}", name=f"at{c}")
               for c in range(2)] if use_at else None)
        wo_sb = [persist.tile([128, EMBED], F16, tag=f"wo{c}", name=f"wo{c}")
                 for c in range(2)]
        for c in range(2):
            nc.sync.dma_start(out=wo_sb[c][:], in_=wo_d[c * 128:(c + 1) * 128, :])

        # ---------------- Phase A: projections ----------------
        with ExitStack() as actx:
            apool = actx.enter_context(tc.tile_pool(name="ain", bufs=1))
            xkv = [apool.tile([128, S], F16, tag=f"xkv{e}", name=f"xkv{e}")
                   for e in range(NE)]
            wk_sb = [apool.tile([128, HC], F16, tag=f"wk{e}", name=f"wk{e}")
                     for e in range(NE)]
            wv_sb = [apool.tile([128, HC], F16, tag=f"wv{e}", name=f"wv{e}")
                     for e in range(NE)]
            xq = [apool.tile([128, T], F16, tag=f"xq{e}", name=f"xq{e}")
                  for e in range(NE)]
            wq_sb = [apool.tile([128, HC], F16, tag=f"wq{e}", name=f"wq{e}")
                     for e in range(NE)]

            for e in range(NE):
                nc.sync.dma_start(out=xkv[e][:], in_=xkv_d[e * 128:(e + 1) * 128, :])
            for e in range(NE):
                nc.sync.dma_start(out=wk_sb[e][:], in_=wk_d[e * 128:(e + 1) * 128, :])
                nc.sync.dma_start(out=wv_sb[e][:], in_=wv_d[e * 128:(e + 1) * 128, :])
            for e in range(NE):
                nc.sync.dma_start(out=xq[e][:], in_=xq_d[e * 128:(e + 1) * 128, :])
                nc.sync.dma_start(out=wq_sb[e][:], in_=wq_d[e * 128:(e + 1) * 128, :])

            # softmax-denominator ones column per v-head
            for s in range(NS):
                v3 = vv[s][:].rearrange("p (h c) -> p h c", c=65)
                nc.vector.memset(v3[:, :, 64:65], 1.0)

            ppool = actx.enter_context(
                tc.tile_pool(name="ppsum", bufs=4, space="PSUM"))

            # V projection: V[s,:] per 128-row s-chunk
            for s in range(NS):
                ps = ppool.tile([128, HC], F32, tag="pj", name=f"vps{s}")
                for e in range(NE):
                    nc.tensor.matmul(ps[:], lhsT=xkv[e][:, s * 128:(s + 1) * 128],
                                     rhs=wv_sb[e][:],
                                     start=(e == 0), stop=(e == NE - 1))
                nc.vector.tensor_copy(
                    vv[s][:].rearrange("p (h c) -> p h c", c=65)[:, :, 0:64],
                    ps[:].rearrange("p (h c) -> p h c", c=64))

            # K^T and Q^T projections: [256, S] as 2 partition chunks
            for w_sb, x_sb, dst in ((wk_sb, xkv, kt), (wq_sb, xq, qt)):
                for m in range(2):
                    for n in range(4):
                        ps = ppool.tile([128, 512], F32, tag="pj",
                                        name=f"qkps{m}_{n}")
                        for e in range(NE):
                            nc.tensor.matmul(
                                ps[:],
                                lhsT=w_sb[e][:, m * 128:(m + 1) * 128],
                                rhs=x_sb[e][:, n * 512:(n + 1) * 512],
                                start=(e == 0), stop=(e == NE - 1))
                        nc.vector.tensor_copy(dst[m][:, n * 512:(n + 1) * 512],
                                              ps[:])

        if "B" not in phases:
            # debug: dump QT/KT straight to the output
            for m in range(2):
                st = persist.tile([128, T], F32, tag=f"dbg{m}",
                                  name=f"dbg{m}")
                nc.vector.tensor_copy(st[:], qt[m][:])
                nc.sync.dma_start(out=out_d[m * 128:(m + 1) * 128, :],
                                  in_=st[:])
                st2 = persist.tile([128, T], F32, tag=f"dbg2{m}",
                                   name=f"dbg2{m}")
                nc.vector.tensor_copy(st2[:], kt[m][:])
                nc.sync.dma_start(out=out_d[(2 + m) * 128:(3 + m) * 128, :],
                                  in_=st2[:])

        # ---------------- Phase B: attention ----------------
        with ExitStack() as bctx:
          if "B" in phases:
            sc_pool = bctx.enter_context(
                tc.tile_pool(name="scps", bufs=2, space="PSUM"))
            av_pool = bctx.enter_context(
                tc.tile_pool(name="avps", bufs=1, space="PSUM"))
            pt_pool = bctx.enter_context(tc.tile_pool(name="pt", bufs=2))
            ep_pool = bctx.enter_context(tc.tile_pool(name="ep", bufs=2))

            for p in range(DBG_B_PAIRS):
                for th in range(2):
                    av = av_pool.tile([128, 1536], F32, tag="av",
                                      name=f"av{p}_{th}")
                    for s in range(NS):
                        pts = []
                        for hi in range(2):
                            h = 2 * p + hi
                            po = (h % 4) * 32
                            ktile = kt[h // 4]
                            qtile = qt[h // 4]
                            sc = sc_pool.tile([128, 1024], F32, tag="sc",
                                              name=f"sc{p}_{th}_{s}_{hi}")
                            for nj in range(2):
                                c0 = th * 1024 + nj * 512
                                nc.tensor.matmul(
                                    sc[:, nj * 512:(nj + 1) * 512],
                                    lhsT=ktile[po:po + 32, s * 128:(s + 1) * 128],
                                    rhs=qtile[po:po + 32, c0:c0 + 512],
                                    start=True, stop=True,
                                    tile_position=(po, 0))
                            pt = pt_pool.tile([128, 1024], F16, tag=f"pt{hi}",
                                              name=f"pt{p}_{th}_{s}_{hi}")
                            nc.scalar.activation(pt[:], sc[:], AF.Exp)
                            pts.append(pt)
                        for hi in range(2 if DBG_B_AV else 0):
                            for t in range(8):
                                i = hi * 8 + t
                                off = _av_off(i)
                                # start=True zeroes the whole 2KB PSUM bank:
                                # only the first slot per bank starts; later
                                # slots' first writes hit still-pending-zero
                                # bytes and overwrite. stop on the last slot
                                # per bank.
                                first_in_bank = (i % 7 == 0)
                                last_in_bank = i in (6, 13, 15)
                                nc.tensor.matmul(
                                    av[:, off:off + 65],
                                    lhsT=pts[hi][:, t * 128:(t + 1) * 128],
                                    rhs=vv[s][:, p * 65:(p + 1) * 65],
                                    start=(s == 0 and first_in_bank),
                                    stop=(s == NS - 1 and last_in_bank),
                                    skip_group_check=True)

                    # epilogue: combine pair, RMS-norm scale, transpose
                    if not (DBG_B_AV and DBG_B_EPI):
                        continue
                    rec = ep_pool.tile([128, 16], F32, tag="rec",
                                       name=f"rec{p}_{th}")
                    sums_a = (av[:, 0:1024]
                              .rearrange("p (a x) -> p a x", a=2)
                              [:, :, 0:455]
                              .rearrange("p a (b c) -> p a b c", c=65)
                              [:, :, :, 64:65])
                    nc.vector.tensor_copy(
                        rec[:, 0:14].rearrange("p (a b c) -> p a b c",
                                               b=7, c=1),
                        sums_a)
                    sums_b = (av[:, 1024:1154]
                              .rearrange("p (b c) -> p b c", c=65)
                              [:, :, 64:65])
                    nc.vector.tensor_copy(
                        rec[:, 14:16].rearrange("p (b c) -> p b c", c=1),
                        sums_b)
                    if DBG_B_EPI_LEVEL < 2:
                        continue
                    nc.vector.reciprocal(rec[:], rec[:])
                    rec2l = ep_pool.tile([128, 8], F32, tag="rec2l",
                                         name=f"rec2l{p}_{th}")
                    nc.vector.tensor_scalar_mul(rec2l[:], rec[:, 8:16], -lam)

                    if DBG_B_EPI_LEVEL < 3:
                        continue
                    ssq = ep_pool.tile([128, 8], F32, tag="ssq",
                                       name=f"ssq{p}_{th}")
                    attns = []
                    for t in range(8):
                        o1 = av[:, _av_off(t):_av_off(t) + 64]
                        o2 = av[:, _av_off(8 + t):_av_off(8 + t) + 64]
                        tmp = ep_pool.tile([128, 64], F32, tag="tmp",
                                           name=f"tmp{p}_{th}_{t}")
                        nc.vector.tensor_scalar_mul(tmp[:], o1, rec[:, t:t + 1])
                        attn = ep_pool.tile([128, 64], F32, tag="attn", bufs=8,
                                            name=f"attn{p}_{th}_{t}")
                        nc.vector.scalar_tensor_tensor(
                            attn[:], o2, rec2l[:, t:t + 1], tmp[:],
                            op0=OP.mult, op1=OP.add)
                        if DBG_B_EPI_LEVEL >= 4:
                            sq = ep_pool.tile([128, 64], F32, tag="sq",
                                              name=f"sq{p}_{th}_{t}")
                            nc.vector.tensor_mul(sq[:], attn[:], attn[:])
                            nc.vector.reduce_sum(ssq[:, t:t + 1], sq[:],
                                                 axis=mybir.AxisListType.X)
                        attns.append(attn)

                    if DBG_B_EPI_LEVEL < 5:
                        continue
                    # rsqrt(mean+eps) = exp(-0.5*ln(ssq/64+eps)); Ln and Exp
                    # share one ACT table set so no table reloads occur.
                    lnt = ep_pool.tile([128, 8], F32, tag="lnt",
                                       name=f"lnt{p}_{th}")
                    nc.scalar.activation(lnt[:], ssq[:], AF.Ln,
                                         scale=1.0 / 64.0, bias=eps_sb[:])
                    rinv = ep_pool.tile([128, 8], F32, tag="rinv",
                                        name=f"rinv{p}_{th}")
                    nc.scalar.activation(rinv[:], lnt[:], AF.Exp, scale=-0.5)

                    po = (p % 2) * 64
                    for t in range(8 if DBG_B_TRANS else 0):
                        a16 = ep_pool.tile([128, 64], F16, tag="a16",
                                           name=f"a16_{p}_{th}_{t}")
                        nc.vector.tensor_scalar_mul(a16[:], attns[t][:],
                                                    rinv[:, t:t + 1])
                        tp = sc_pool.tile([128, 128], F16, tag="sc",
                                          name=f"tp{p}_{th}_{t}")
                        nc.tensor.transpose(tp[po:po + 64, :], a16[:], ident[:])
                        nc.vector.tensor_copy(
                            at[p // 2][po:po + 64,
                                       th * 1024 + t * 128:
                                       th * 1024 + (t + 1) * 128],
                            tp[po:po + 64, :])

        if "C" not in phases and "B" in phases and use_at:
            for c in range(2):
                st = persist.tile([128, T], F32, tag=f"dbgc{c}",
                                  name=f"dbgc{c}")
                nc.vector.tensor_copy(st[:], at[c][:])
                nc.sync.dma_start(out=out_d[c * 128:(c + 1) * 128, :],
                                  in_=st[:])

        # ---------------- Phase C: output projection ----------------
        with ExitStack() as cctx:
          if "C" in phases:
            cpool = cctx.enter_context(
                tc.tile_pool(name="cpsum", bufs=2, space="PSUM"))
            spool = cctx.enter_context(tc.tile_pool(name="cst", bufs=3))
            for e in range(8):
                for nt in range(4):
                    ps = cpool.tile([128, 512], F32, tag="op",
                                    name=f"ops{e}_{nt}")
                    for c in range(2):
                        nc.tensor.matmul(
                            ps[:],
                            lhsT=wo_sb[c][:, e * 128:(e + 1) * 128],
                            rhs=at[c][:, nt * 512:(nt + 1) * 512],
                            start=(c == 0), stop=(c == 1))
                    st = spool.tile([128, 512], F32, tag="st",
                                    name=f"st{e}_{nt}")
                    nc.vector.tensor_copy(st[:], ps[:])
                    nc.sync.dma_start(
                        out=out_d[e * 128:(e + 1) * 128,
                                  nt * 512:(nt + 1) * 512],
                        in_=st[:])

    nc.compile()
    return nc


def _run(inputs, trace=False, trace_cores=None):
    q = np.asarray(inputs["query_x"], np.float32)
    kv = np.asarray(inputs["kv_x"], np.float32)
    Wq = np.asarray(inputs["Wq"], np.float32)
    Wk = np.asarray(inputs["Wk"], np.float32)
    Wv = np.asarray(inputs["Wv"], np.float32)
    Wo = np.asarray(inputs["Wo"], np.float32)
    subln_w = np.asarray(inputs["subln_w"], np.float32)

    lam1 = np.exp(np.sum(np.asarray(inputs["lambda_q1"], np.float32)
                         * np.asarray(inputs["lambda_k1"], np.float32),
                         dtype=np.float32))
    lam2 = np.exp(np.sum(np.asarray(inputs["lambda_q2"], np.float32)
                         * np.asarray(inputs["lambda_k2"], np.float32),
                         dtype=np.float32))
    lam = float(np.float32(lam1 - lam2 + np.float32(LAMBDA_INIT)))

    key = round(lam, 12)
    if key not in _cache:
        _cache[key] = _build(lam)
    nc = _cache[key]

    scaling = np.float32(D ** -0.5)
    wo_scale = (np.tile(subln_w, 4) * np.float32(1.0 - LAMBDA_INIT))
    in_maps = []
    for core in range(8):
        b, g = divmod(core, 4)
        sl = slice(g * HC, (g + 1) * HC)
        in_maps.append({
            "xq_t": np.ascontiguousarray(q[b].T).astype(np.float16),
            "xkv_t": np.ascontiguousarray(kv[b].T).astype(np.float16),
            "wq": (Wq[:, sl] * scaling).astype(np.float16),
            "wk": Wk[:, sl].astype(np.float16),
            "wv": Wv[:, sl].astype(np.float16),
            "wo": (Wo[sl, :] * wo_scale[:, None]).astype(np.float16),
        })

    res = run_bass_kernel_spmd(nc, in_maps, list(range(8)), trace=False)

    out = np.zeros((2, T, EMBED), np.float32)
    for core in range(8):
        out[core // 4] += res.results[core]["out_t"].T
    return out, res


def kernel(**inputs):
    out, _ = _run(inputs, trace=False)
    return out


def _run_timed(inputs, iters=30):
    """Like _run but keeps inputs device-resident and wall-clocks repeated
    NEFF executions (no profiler hook available under this axon client).
    Returns (out, per_iter_ns)."""
    import time
    import jax
    import jax.numpy as jnp
    from jax.experimental.shard_map import shard_map
    from jax.sharding import Mesh, NamedSharding, PartitionSpec
    from concourse import bass2jax, mybir as mb

    q = np.asarray(inputs["query_x"], np.float32)
    kv = np.asarray(inputs["kv_x"], np.float32)
    Wq = np.asarray(inputs["Wq"], np.float32)
    Wk = np.asarray(inputs["Wk"], np.float32)
    Wv = np.asarray(inputs["Wv"], np.float32)
    Wo = np.asarray(inputs["Wo"], np.float32)
    subln_w = np.asarray(inputs["subln_w"], np.float32)
    lam1 = np.exp(np.sum(np.asarray(inputs["lambda_q1"], np.float32)
                         * np.asarray(inputs["lambda_k1"], np.float32),
                         dtype=np.float32))
    lam2 = np.exp(np.sum(np.asarray(inputs["lambda_q2"], np.float32)
                         * np.asarray(inputs["lambda_k2"], np.float32),
                         dtype=np.float32))
    lam = float(np.float32(lam1 - lam2 + np.float32(LAMBDA_INIT)))
    key = round(lam, 12)
    if key not in _cache:
        _cache[key] = _build(lam)
    nc = _cache[key]

    scaling = np.float32(D ** -0.5)
    wo_scale = (np.tile(subln_w, 4) * np.float32(1.0 - LAMBDA_INIT))
    in_maps = []
    for core in range(8):
        b, g = divmod(core, 4)
        sl = slice(g * HC, (g + 1) * HC)
        in_maps.append({
            "xq_t": np.ascontiguousarray(q[b].T).astype(np.float16),
            "xkv_t": np.ascontiguousarray(kv[b].T).astype(np.float16),
            "wq": (Wq[:, sl] * scaling).astype(np.float16),
            "wk": Wk[:, sl].astype(np.float16),
            "wv": Wv[:, sl].astype(np.float16),
            "wo": (Wo[sl, :] * wo_scale[:, None]).astype(np.float16),
        })

    bass2jax.install_neuronx_cc_hook()
    n_cores = 8
    partition_name = (nc.partition_id_tensor.name
                      if nc.partition_id_tensor else None)
    in_names, out_names, out_avals, zero_outs = [], [], [], []
    for alloc in nc.m.functions[0].allocations:
        if not isinstance(alloc, mb.MemoryLocationSet):
            continue
        name = alloc.memorylocations[0].name
        if alloc.kind == "ExternalInput":
            if name != partition_name:
                in_names.append(name)
        elif alloc.kind == "ExternalOutput":
            out_names.append(name)
            shape = tuple(alloc.tensor_shape)
            dtype = mb.dt.np(alloc.dtype)
            out_avals.append(jax.core.ShapedArray(shape, dtype))
            zero_outs.append(np.zeros(shape, dtype))
    n_params = len(in_names)
    all_names = in_names + out_names
    if partition_name is not None:
        all_names = all_names + [partition_name]

    def _body(*args):
        operands = list(args)
        if partition_name is not None:
            operands.append(bass2jax.partition_id_tensor())
        outs = bass2jax._bass_exec_p.bind(
            *operands,
            out_avals=tuple(out_avals),
            in_names=tuple(all_names),
            out_names=tuple(out_names),
            lowering_input_output_aliases=(),
            sim_require_finite=True,
            sim_require_nnan=True,
            nc=nc,
        )
        return tuple(outs)

    devices = jax.devices()[:n_cores]
    mesh = Mesh(np.asarray(devices), ("core",))
    spec = NamedSharding(mesh, PartitionSpec("core"))
    n_outs = len(out_names)
    sharded = jax.jit(
        shard_map(_body, mesh=mesh,
                  in_specs=(PartitionSpec("core"),) * (n_params + n_outs),
                  out_specs=(PartitionSpec("core"),) * n_outs,
                  check_rep=False),
        keep_unused=True)

    concat_in = [
        jax.device_put(
            np.concatenate([in_maps[c][nm] for c in range(n_cores)], axis=0),
            spec)
        for nm in in_names
    ]
    concat_zeros = [
        jax.device_put(np.zeros((n_cores * z.shape[0], *z.shape[1:]), z.dtype),
                       spec)
        for z in zero_outs
    ]

    out_arrs = sharded(*concat_in, *concat_zeros)  # compile + first run
    jax.block_until_ready(out_arrs)

    for _ in range(3):  # warmup
        jax.block_until_ready(sharded(*concat_in, *concat_zeros))

    best = None
    for _rep in range(3):
        t0 = time.perf_counter()
        rs = [sharded(*concat_in, *concat_zeros) for _ in range(iters)]
        jax.block_until_ready(rs)
        t1 = time.perf_counter()
        per = (t1 - t0) / iters
        best = per if best is None else min(best, per)

    out = np.zeros((2, T, EMBED), np.float32)
    full = np.asarray(out_arrs[0]).reshape(n_cores, EMBED, T)
    for core in range(8):
        out[core // 4] += full[core].T
    return out, best * 1e9



# revision 2
# speedup vs baseline: 2.4097x; 2.4097x over previous
"""CrossMultiheadDiffAttn Trainium2 kernel, v2 (pipelined).

Sharding: 8 cores = 2 batches x 4 head-groups. Each core handles one batch
and 8 of the 32 q/k attention heads (= 4 of the 16 output heads / v-heads).
q/k/v projections column-sharded, out projection row-sharded; the 4 partial
[T, E] outputs per batch are summed on the host.

ACT exp is the bottleneck engine (~266us busy of 33.5M exps); everything
else is arranged to hide under it:
  - single activation table set (natural_log_exp_and_others) for both the
    softmax Exp and the epilogue Ln/Exp: no ACT table reloads.
  - one big DMA per input tensor, split across both HWDGE queues (SP +
    ACT), so the DMA-issue serialization doesn't gate the first exp.
  - globally software-pipelined emission: scores for slot k+1 are emitted
    before AV for slot k so the ACT exp stream never waits on PE; the
    projection chunks (vv/kt1/qt1/qt0-hi) are dripped into early slots;
    the epilogue of unit u-1 is emitted inside unit u; half of the output
    projection (th0) is prestaged into late-unit slots.
  - av PSUM accumulators evacuated with one bulk DVE copy so the next
    unit's AV matmuls only wait ~1.3us instead of the whole epilogue.
  - attn transposes on the DMA xbar (4 big transposes, 3D output AP).
  - fp16 output (halves output DMA + PSUM->SBUF copy cost).

Per-core phases (all matmul operands fp16, fp32 PSUM accumulation):
  A) V = xkv @ Wv (per s-chunk), K^T = Wk^T @ xkv^T, Q^T = Wq^T @ xq^T
     (host pre-transposes x and pre-scales Wq by D^-0.5).
  B) per unit (head-pair, t-half): per s-chunk: scoresT[s,t] = K_h Q_h^T
     on PE (row-tiled quadrants, 2 heads) -> exp on ACT (no max
     subtraction; scores ~ N(0,1)) -> P^T fp16 -> attn accum
     [t,64+1] = P^T.T @ [V|1] on PE (ones column gives the softmax
     denominators). Epilogue: combine o1/s1 - lam*o2/s2, RMSNorm scale
     via rsqrt(m) = exp(-0.5*ln(m)), DMA-xbar transpose into attnT.
  C) out^T = Wo'^T @ attnT (Wo' pre-scaled by subln_w*(1-lambda_init)),
     DMA out as [E, T] fp16; host transposes and reduces.
"""

import math

import numpy as np

import bass_rust as _bass_rust
import concourse.bacc as bacc
import concourse.bass as bass
import concourse.mybir as mybir
import concourse.tile as tile
from concourse.bass_utils import run_bass_kernel_spmd
from concourse.hw_specs import get_activation_tables
from contextlib import ExitStack

F32 = mybir.dt.float32
F16 = mybir.dt.float16
AF = mybir.ActivationFunctionType
OP = mybir.AluOpType

EMBED = 1024
HEADS = 16
DEPTH = 12
D = 32                       # head dim of q/k heads
LAMBDA_INIT = 0.8 - 0.6 * math.exp(-0.3 * DEPTH)
T = 2048
S = 2048
HC = 256                     # per-core projection columns (8 qk heads * 32)
NE = EMBED // 128            # 8 contraction chunks
NS = S // 128                # 16 s chunks
PAIRS = 4                    # head pairs per core
EPS = 1e-5

_cache = {}

# th-major unit order: both groups' th0 halves finish by unit 3 so the
# th0 half of the output projection can prestage into units 4-7.
UNITS = [(0, 0), (1, 0), (2, 0), (3, 0), (0, 1), (1, 1), (2, 1), (3, 1)]
NSLOT = 8 * NS


def _av_off(i):
    # 16 psum accumulators of 65 f32 each, packed 7 per 2KB bank (no
    # matmul output may straddle a bank boundary).
    return (i // 7) * 512 + (i % 7) * 65


AV_USED = 2 * 512 + 2 * 65   # f32 elems of the av tile actually populated


class _BaccOneActSet(bacc.Bacc):
    """Bacc whose activation-table pass only considers the set holding both
    exp and ln, so one table load serves the whole kernel."""

    def insert_act_table_loads(self):
        has_activation = any(
            isinstance(i, mybir.InstActivation)
            for b in self.main_func.blocks
            for i in b.instructions
        )
        if not has_activation:
            return
        tables = []
        for name, fns in get_activation_tables(self.m.arch).items():
            if name == "natural_log_exp_and_others":
                tables.append((name, fns))
            else:
                tables.append((name, set()))
        _bass_rust.insert_act_table_loads(self, tables)


def _build(lam: float, rounds: int = 1):
    # rounds > 1 repeats the (idempotent) attention stream inside one NEFF;
    # used only to calibrate real per-round hardware time above the
    # dispatch-RPC noise floor.
    nc = _BaccOneActSet("TRN2", target_bir_lowering=False, debug=False,
                        enable_asserts=False, num_devices=8)

    xq_d = nc.dram_tensor("xq_t", [EMBED, T], F16, kind="ExternalInput").ap()
    xkv_d = nc.dram_tensor("xkv_t", [EMBED, S], F16, kind="ExternalInput").ap()
    wq_d = nc.dram_tensor("wq", [EMBED, HC], F16, kind="ExternalInput").ap()
    wk_d = nc.dram_tensor("wk", [EMBED, HC], F16, kind="ExternalInput").ap()
    wv_d = nc.dram_tensor("wv", [EMBED, HC], F16, kind="ExternalInput").ap()
    wo_d = nc.dram_tensor("wo", [HC, EMBED], F16, kind="ExternalInput").ap()
    out_d = nc.dram_tensor("out_t", [EMBED, T], F16, kind="ExternalOutput").ap()

    with tile.TileContext(nc) as tc, ExitStack() as ctx:
        persist = ctx.enter_context(tc.tile_pool(name="persist", bufs=1))
        eps_sb = persist.tile([128, 1], F32, tag="eps", name="eps")
        nc.vector.memset(eps_sb[:], EPS)

        qt = [persist.tile([128, T], F16, tag=f"qt{m}", name=f"qt{m}")
              for m in range(2)]
        kt = [persist.tile([128, S], F16, tag=f"kt{m}", name=f"kt{m}")
              for m in range(2)]
        vv = [persist.tile([128, 4 * 65], F16, tag=f"vv{s}", name=f"vv{s}")
              for s in range(NS)]
        at = [persist.tile([128, T], F16, tag=f"at{c}", name=f"at{c}")
              for c in range(2)]
        wo_sb = persist.tile([128, 2, EMBED], F16, tag="wo", name="wo_sb")
        # output staging: one tile per (e-chunk, t-half)
        st = [persist.tile([128, 1024], F16, tag=f"st{e}_{h}",
                           name=f"st{e}_{h}")
              for e in range(8) for h in range(2)]

        apool = ctx.enter_context(tc.tile_pool(name="ain", bufs=1))
        xkv_t = apool.tile([128, NE, S], F16, tag="xkv", name="xkv_t_sb")
        xq_t = apool.tile([128, NE, T], F16, tag="xq", name="xq_t_sb")
        wk_sb = apool.tile([128, NE, HC], F16, tag="wk", name="wk_sb")
        wv_sb = apool.tile([128, NE, HC], F16, tag="wv", name="wv_sb")
        wq_sb = apool.tile([128, NE, HC], F16, tag="wq", name="wq_sb")

        # few big DMAs, split across the two HWDGE queues; x tensors in two
        # halves so the first half's projection matmuls warm the PE while
        # the second half transfers. wv trails (vv chunks run per-slot).
        xkv_r = xkv_d.rearrange("(a p) t -> p a t", p=128)
        xq_r = xq_d.rearrange("(a p) t -> p a t", p=128)
        nc.sync.dma_start(out=wk_sb[:],
                          in_=wk_d.rearrange("(a p) c -> p a c", p=128))
        nc.sync.dma_start(out=wq_sb[:],
                          in_=wq_d.rearrange("(a p) c -> p a c", p=128))
        nc.sync.dma_start(out=xkv_t[:, 0:4, :], in_=xkv_r[:, 0:4, :])
        nc.sync.dma_start(out=xq_t[:, 0:4, :], in_=xq_r[:, 0:4, :])
        nc.sync.dma_start(out=xkv_t[:, 4:8, :], in_=xkv_r[:, 4:8, :])
        nc.sync.dma_start(out=xq_t[:, 4:8, :], in_=xq_r[:, 4:8, :])
        nc.sync.dma_start(out=wv_sb[:],
                          in_=wv_d.rearrange("(a p) c -> p a c", p=128))
        nc.scalar.dma_start(out=wo_sb[:],
                            in_=wo_d.rearrange("(a p) c -> p a c", p=128))

        # softmax-denominator ones column per v-head
        for s in range(NS):
            v3 = vv[s][:].rearrange("p (h c) -> p h c", c=65)
            nc.vector.memset(v3[:, :, 64:65], 1.0)

        # epilogue SBUF pools
        ep = ctx.enter_context(tc.tile_pool(name="ep", bufs=2))
        pt_pool = ctx.enter_context(tc.tile_pool(name="pt", bufs=2))

        with ExitStack() as bctx:
            bpool = bctx.enter_context(
                tc.tile_pool(name="bpsum", bufs=1, space="PSUM"))

            # ---- projection chunk emitters ----
            # tag "pj" (1 bank) for chunks dripped into attention slots;
            # the prefix passes tag="sc" (2 bufs) so chunk k+1's matmuls
            # overlap chunk k's copy while the sc ring is otherwise idle.
            def qk_chunk(dst_pair, w_sb, x_sb, m, n, nm, half=None,
                         tag="pj", bufs=1):
                # [128, 512] chunk of dst_pair[m] at columns n*512..; half
                # splits the 8-matmul accumulation into two emissions.
                es = range(NE) if half is None else (
                    range(4) if half == 0 else range(4, NE))
                if half in (None, 0):
                    full = bpool.tile([128, 512 if tag == "pj" else 1024],
                                      F32, tag=tag, bufs=bufs, name=f"pj{nm}")
                    ps = full[:, 0:512]
                    qk_chunk.ps = ps
                else:
                    ps = qk_chunk.ps
                for e in es:
                    nc.tensor.matmul(
                        ps[:],
                        lhsT=w_sb[:, e, m * 128:(m + 1) * 128],
                        rhs=x_sb[:, e, n * 512:(n + 1) * 512],
                        start=(e == 0), stop=(e == NE - 1))
                if half in (None, 1):
                    nc.vector.tensor_copy(
                        dst_pair[m][:, n * 512:(n + 1) * 512], ps[:])

            def vv_chunk(s):
                ps = bpool.tile([128, 512], F32, tag="pj", bufs=1,
                                name=f"vps{s}")
                for e in range(NE):
                    nc.tensor.matmul(ps[:, 0:HC],
                                     lhsT=xkv_t[:, e, s * 128:(s + 1) * 128],
                                     rhs=wv_sb[:, e, :],
                                     start=(e == 0), stop=(e == NE - 1))
                nc.vector.tensor_copy(
                    vv[s][:].rearrange("p (h c) -> p h c", c=65)[:, :, 0:64],
                    ps[:, 0:HC].rearrange("p (h c) -> p h c", c=64))

            # ---- phase-C tile emitter (shares psum tag "pj") ----
            def phc_tile(e, nt, pool=None, copy_eng="dve"):
                ps = (pool or bpool).tile([128, 512], F32, tag="pj", bufs=1,
                                          name=f"ops{e}_{nt}")
                for c in range(2):
                    nc.tensor.matmul(
                        ps[:],
                        lhsT=wo_sb[:, c, e * 128:(e + 1) * 128],
                        rhs=at[c][:, nt * 512:(nt + 1) * 512],
                        start=(c == 0), stop=(c == 1))
                dst = st[e * 2 + nt // 2][:, (nt % 2) * 512:(nt % 2) * 512 + 512]
                if copy_eng == "act":
                    nc.scalar.activation(dst, ps[:], AF.Copy)
                else:
                    nc.vector.tensor_copy(dst, ps[:])
                nc.sync.dma_start(
                    out=out_d[e * 128:(e + 1) * 128,
                              nt * 512:(nt + 1) * 512],
                    in_=st[e * 2 + nt // 2][:, (nt % 2) * 512:
                                            (nt % 2) * 512 + 512])

            # ---- attention slot machinery ----
            av_tiles = {}
            avsbs = {}
            a16g = {}

            def emit_sc_exp(k):
                u, s = divmod(k, NS)
                p, th = UNITS[u]
                m = p // 2
                pts = []
                for hi in range(2):
                    h = 2 * p + hi
                    po = (h % 4) * 32
                    sc = bpool.tile([128, 1024], F32, tag="sc", bufs=2,
                                    name=f"sc{u}_{s}_{hi}")
                    for nj in range(2):
                        c0 = th * 1024 + nj * 512
                        nc.tensor.matmul(
                            sc[:, nj * 512:(nj + 1) * 512],
                            lhsT=kt[m][po:po + 32, s * 128:(s + 1) * 128],
                            rhs=qt[m][po:po + 32, c0:c0 + 512],
                            start=True, stop=True,
                            tile_position=(po, 0))
                    pt = pt_pool.tile([128, 1024], F16, tag=f"pt{hi}",
                                      name=f"pt{u}_{s}_{hi}")
                    nc.scalar.activation(pt[:], sc[:], AF.Exp)
                    pts.append(pt)
                return pts

            def emit_av(k, pts):
                u, s = divmod(k, NS)
                p, th = UNITS[u]
                if s == 0:
                    av_tiles[u] = bpool.tile([128, 1536], F32, tag="av",
                                             bufs=1, name=f"av{u}")
                av = av_tiles[u]
                for hi in range(2):
                    for t in range(8):
                        i = hi * 8 + t
                        off = _av_off(i)
                        # start=True zeroes the whole 2KB PSUM bank: only
                        # the first slot per bank starts; stop on the last
                        # slot per bank.
                        first_in_bank = (i % 7 == 0)
                        last_in_bank = i in (6, 13, 15)
                        nc.tensor.matmul(
                            av[:, off:off + 65],
                            lhsT=pts[hi][:, t * 128:(t + 1) * 128],
                            rhs=vv[s][:, p * 65:(p + 1) * 65],
                            start=(s == 0 and first_in_bank),
                            stop=(s == NS - 1 and last_in_bank),
                            skip_group_check=True)

            def epi_copy(u):
                # bulk-evacuate av so the next unit's AV matmuls can start
                avsb = ep.tile([128, 1160], F32, tag="avsb", bufs=2,
                               name=f"avsb{u}")
                nc.vector.tensor_copy(avsb[:, 0:AV_USED],
                                      av_tiles[u][:, 0:AV_USED])
                avsbs[u] = avsb

            def epi_math(u, act_free=False, do_transpose=True):
                # act_free: ACT has no exp stream left (final unit), so the
                # square+sum steps can run there in parallel with DVE.
                p, th = UNITS[u]
                avsb = avsbs[u]
                rec = ep.tile([128, 16], F32, tag="rec", name=f"rec{u}")
                sums_a = (avsb[:, 0:1024]
                          .rearrange("p (a x) -> p a x", a=2)
                          [:, :, 0:455]
                          .rearrange("p a (b c) -> p a b c", c=65)
                          [:, :, :, 64:65])
                nc.vector.tensor_copy(
                    rec[:, 0:14].rearrange("p (a b c) -> p a b c", b=7, c=1),
                    sums_a)
                sums_b = (avsb[:, 1024:1154]
                          .rearrange("p (b c) -> p b c", c=65)
                          [:, :, 64:65])
                nc.vector.tensor_copy(
                    rec[:, 14:16].rearrange("p (b c) -> p b c", c=1),
                    sums_b)
                nc.vector.reciprocal(rec[:], rec[:])
                rec2l = ep.tile([128, 8], F32, tag="rec2l", name=f"rec2l{u}")
                nc.vector.tensor_scalar_mul(rec2l[:], rec[:, 8:16], -lam)

                ssq = ep.tile([128, 8], F32, tag="ssq", name=f"ssq{u}")
                attns = []
                for t in range(8):
                    o1 = avsb[:, _av_off(t):_av_off(t) + 64]
                    o2 = avsb[:, _av_off(8 + t):_av_off(8 + t) + 64]
                    tmp = ep.tile([128, 64], F32, tag="tmp",
                                  name=f"tmp{u}_{t}")
                    nc.vector.tensor_scalar_mul(tmp[:], o1, rec[:, t:t + 1])
                    attn = ep.tile([128, 64], F32, tag="attn", bufs=16,
                                   name=f"attn{u}_{t}")
                    nc.vector.scalar_tensor_tensor(
                        attn[:], o2, rec2l[:, t:t + 1], tmp[:],
                        op0=OP.mult, op1=OP.add)
                    if act_free:
                        sq = ep.tile([128, 64], F32, tag="sq",
                                     name=f"sq{u}_{t}")
                        nc.scalar.activation(sq[:], attn[:], AF.Square,
                                             accum_out=ssq[:, t:t + 1])
                    else:
                        sq = ep.tile([128, 64], F32, tag="sq",
                                     name=f"sq{u}_{t}")
                        nc.vector.tensor_mul(sq[:], attn[:], attn[:])
                        nc.vector.reduce_sum(ssq[:, t:t + 1], sq[:],
                                             axis=mybir.AxisListType.X)
                    attns.append(attn)

                # rsqrt(mean+eps) = exp(-0.5*ln(ssq/64+eps)); Ln and Exp
                # live in one ACT table set (forced) so no table reloads.
                lnt = ep.tile([128, 8], F32, tag="lnt", name=f"lnt{u}")
                nc.scalar.activation(lnt[:], ssq[:], AF.Ln,
                                     scale=1.0 / 64.0, bias=eps_sb[:])
                rinv = ep.tile([128, 8], F32, tag="rinv", name=f"rinv{u}")
                nc.scalar.activation(rinv[:], lnt[:], AF.Exp, scale=-0.5)

                # normalized attn, fp16, packed for the DMA-xbar transpose:
                # [t-chunk(8) x 128] where cols 0:64 = even pair of the
                # group, 64:128 = odd pair. One 3D-output xbar transpose
                # per (group, th) then fills all of at[g][:, th-half].
                g = p // 2
                gk = (g, th)
                if gk not in a16g:
                    a16g[gk] = ep.tile([128, 1024], F16, tag="a16g", bufs=2,
                                       name=f"a16g_{g}_{th}")
                a16 = a16g[gk]
                po = (p % 2) * 64
                for t in range(8):
                    nc.vector.tensor_scalar_mul(
                        a16[:, t * 128 + po:t * 128 + po + 64],
                        attns[t][:], rinv[:, t:t + 1])
                if p % 2 == 1 and do_transpose:
                    # two halves so the output projection can start on the
                    # first t-quarter while the second still transposes
                    for hh in range(2):
                        nc.sync.dma_start(
                            out=at[g][:, th * 1024 + hh * 512:
                                      th * 1024 + (hh + 1) * 512]
                            .rearrange("p (j t) -> p j t", t=128),
                            in_=a16[:, hh * 512:(hh + 1) * 512],
                            transpose=True)

            # ---- deferred work schedule: items per slot ----
            extras = {}
            extras[0] = [(vv_chunk, (0,)), (vv_chunk, (1,))]
            for s in range(2, NS):                    # vv[2..15]
                extras[s - 1] = [(vv_chunk, (s,))]
            ex1 = []
            for m, ns in ((1, range(4)),):            # kt1 n0-3 as halves
                for n in ns:
                    ex1.append((qk_chunk, (kt, wk_sb, xkv_t, m, n,
                                           f"kt1n{n}", 0)))
                    ex1.append((qk_chunk, (kt, wk_sb, xkv_t, m, n,
                                           f"kt1n{n}", 1)))
            for n in range(4):                        # qt1 n0-3 as halves
                ex1.append((qk_chunk, (qt, wq_sb, xq_t, 1, n, f"qt1n{n}", 0)))
                ex1.append((qk_chunk, (qt, wq_sb, xq_t, 1, n, f"qt1n{n}", 1)))
            for n in (2, 3):                          # qt0 n2-3 as halves
                ex1.append((qk_chunk, (qt, wq_sb, xq_t, 0, n, f"qt0n{n}", 0)))
                ex1.append((qk_chunk, (qt, wq_sb, xq_t, 0, n, f"qt0n{n}", 1)))
            for j, item in enumerate(ex1):
                extras.setdefault(16 + j, []).append(item)   # slots 16..35
            for j in range(16):                       # phC th0: slots 72..
                e, nt = j // 2, j % 2
                extras.setdefault(72 + 2 * j, []).append((phc_tile, (e, nt)))

            # ---- prefix: minimum needed for slot 0 ----
            # kt0 chunks pipeline through the (idle) sc ring; qt0 n0/n1
            # borrow the untouched av banks so they only gate on the xq
            # DMA, not on the kt chunk chain.
            for n in range(4):
                qk_chunk(kt, wk_sb, xkv_t, 0, n, f"kt0n{n}", tag="sc", bufs=2)
            pq = bpool.tile([128, 1536], F32, tag="av", bufs=1, name="pjav")
            for n in range(2):
                ps = pq[:, n * 512:(n + 1) * 512]
                for e in range(NE):
                    nc.tensor.matmul(
                        ps,
                        lhsT=wq_sb[:, e, 0:128],
                        rhs=xq_t[:, e, n * 512:(n + 1) * 512],
                        start=(e == 0), stop=(e == NE - 1))
                nc.vector.tensor_copy(qt[0][:, n * 512:(n + 1) * 512], ps)

            # ---- pipelined slot loop ----
            TOT = rounds * NSLOT
            pts_next = emit_sc_exp(0)
            for gk in range(TOT):
                k, r = gk % NSLOT, gk // NSLOT
                pts = pts_next
                pts_next = (emit_sc_exp((gk + 1) % NSLOT)
                            if gk + 1 < TOT else None)
                for fn, args in extras.get(k, ()):
                    if (k < 64 and r == 0) or (k >= 64 and r == rounds - 1):
                        fn(*args)
                u, s = divmod(k, NS)
                if s == 5 and (u > 0 or r > 0):
                    epi_math((u - 1) % 8,
                             do_transpose=(r == rounds - 1 and u > 0))
                emit_av(k, pts)
                if s == NS - 1:
                    epi_copy(u)
            epi_math(7, act_free=True)

        # ---------------- Phase C tail: th1 half ----------------
        with ExitStack() as cctx:
            cpool = cctx.enter_context(
                tc.tile_pool(name="cpsum", bufs=6, space="PSUM"))
            for j in range(16):
                e, nt = j % 8, 2 + j // 8      # all nt=2 first, then nt=3
                phc_tile(e, nt, pool=cpool,
                         copy_eng="act" if j % 2 else "dve")

    nc.compile()
    return nc


def _prep(inputs):
    q = np.asarray(inputs["query_x"], np.float32)
    kv = np.asarray(inputs["kv_x"], np.float32)
    Wq = np.asarray(inputs["Wq"], np.float32)
    Wk = np.asarray(inputs["Wk"], np.float32)
    Wv = np.asarray(inputs["Wv"], np.float32)
    Wo = np.asarray(inputs["Wo"], np.float32)
    subln_w = np.asarray(inputs["subln_w"], np.float32)

    lam1 = np.exp(np.sum(np.asarray(inputs["lambda_q1"], np.float32)
                         * np.asarray(inputs["lambda_k1"], np.float32),
                         dtype=np.float32))
    lam2 = np.exp(np.sum(np.asarray(inputs["lambda_q2"], np.float32)
                         * np.asarray(inputs["lambda_k2"], np.float32),
                         dtype=np.float32))
    lam = float(np.float32(lam1 - lam2 + np.float32(LAMBDA_INIT)))

    scaling = np.float32(D ** -0.5)
    wo_scale = (np.tile(subln_w, 4) * np.float32(1.0 - LAMBDA_INIT))
    in_maps = []
    for core in range(8):
        b, g = divmod(core, 4)
        sl = slice(g * HC, (g + 1) * HC)
        in_maps.append({
            "xq_t": np.ascontiguousarray(q[b].T).astype(np.float16),
            "xkv_t": np.ascontiguousarray(kv[b].T).astype(np.float16),
            "wq": (Wq[:, sl] * scaling).astype(np.float16),
            "wk": Wk[:, sl].astype(np.float16),
            "wv": Wv[:, sl].astype(np.float16),
            "wo": (Wo[sl, :] * wo_scale[:, None]).astype(np.float16),
        })
    return lam, in_maps


def _run(inputs):
    lam, in_maps = _prep(inputs)
    key = round(lam, 12)
    if key not in _cache:
        _cache[key] = _build(lam)
    nc = _cache[key]

    res = run_bass_kernel_spmd(nc, in_maps, list(range(8)), trace=False)

    out = np.zeros((2, T, EMBED), np.float32)
    for core in range(8):
        out[core // 4] += res.results[core]["out_t"].T.astype(np.float32)
    return out, res


def kernel(**inputs):
    out, _ = _run(inputs)
    return out


def _run_timed(inputs, iters=30):
    """Keeps inputs device-resident and wall-clocks repeated NEFF
    executions (no profiler hook available under this axon client).
    Returns (out, per_iter_ns)."""
    import time
    import jax
    from jax.experimental.shard_map import shard_map
    from jax.sharding import Mesh, NamedSharding, PartitionSpec
    from concourse import bass2jax, mybir as mb

    lam, in_maps = _prep(inputs)
    key = round(lam, 12)
    if key not in _cache:
        _cache[key] = _build(lam)
    nc = _cache[key]

    bass2jax.install_neuronx_cc_hook()
    n_cores = 8
    partition_name = (nc.partition_id_tensor.name
                      if nc.partition_id_tensor else None)
    in_names, out_names, out_avals, zero_outs = [], [], [], []
    for alloc in nc.m.functions[0].allocations:
        if not isinstance(alloc, mb.MemoryLocationSet):
            continue
        name = alloc.memorylocations[0].name
        if alloc.kind == "ExternalInput":
            if name != partition_name:
                in_names.append(name)
        elif alloc.kind == "ExternalOutput":
            out_names.append(name)
            shape = tuple(alloc.tensor_shape)
            dtype = mb.dt.np(alloc.dtype)
            out_avals.append(jax.core.ShapedArray(shape, dtype))
            zero_outs.append(np.zeros(shape, dtype))
    n_params = len(in_names)
    all_names = in_names + out_names
    if partition_name is not None:
        all_names = all_names + [partition_name]

    def _body(*args):
        operands = list(args)
        if partition_name is not None:
            operands.append(bass2jax.partition_id_tensor())
        outs = bass2jax._bass_exec_p.bind(
            *operands,
            out_avals=tuple(out_avals),
            in_names=tuple(all_names),
            out_names=tuple(out_names),
            lowering_input_output_aliases=(),
            sim_require_finite=True,
            sim_require_nnan=True,
            nc=nc,
        )
        return tuple(outs)

    devices = jax.devices()[:n_cores]
    mesh = Mesh(np.asarray(devices), ("core",))
    spec = NamedSharding(mesh, PartitionSpec("core"))
    n_outs = len(out_names)
    sharded = jax.jit(
        shard_map(_body, mesh=mesh,
                  in_specs=(PartitionSpec("core"),) * (n_params + n_outs),
                  out_specs=(PartitionSpec("core"),) * n_outs,
                  check_rep=False),
        keep_unused=True)

    concat_in = [
        jax.device_put(
            np.concatenate([in_maps[c][nm] for c in range(n_cores)], axis=0),
            spec)
        for nm in in_names
    ]
    concat_zeros = [
        jax.device_put(np.zeros((n_cores * z.shape[0], *z.shape[1:]), z.dtype),
                       spec)
        for z in zero_outs
    ]

    out_arrs = sharded(*concat_in, *concat_zeros)  # compile + first run
    jax.block_until_ready(out_arrs)

    for _ in range(3):  # warmup
        jax.block_until_ready(sharded(*concat_in, *concat_zeros))

    best = None
    for _rep in range(3):
        t0 = time.perf_counter()
        rs = [sharded(*concat_in, *concat_zeros) for _ in range(iters)]
        jax.block_until_ready(rs)
        t1 = time.perf_counter()
        per = (t1 - t0) / iters
        best = per if best is None else min(best, per)

    out = np.zeros((2, T, EMBED), np.float32)
    full = np.asarray(out_arrs[0]).reshape(n_cores, EMBED, T)
    for core in range(8):
        out[core // 4] += full[core].T.astype(np.float32)
    return out, best * 1e9
